# revision 3
# baseline (speedup 1.0000x reference)
"""APPNP GNN kernel for 8 Trainium2 NeuronCores.

Strategy (per sharding hint): nodes sharded across 8 cores; edges
partitioned by destination core so segment_sum is local; per
propagation step the (pre-scaled) z shards are AllGathered, then each
core gathers source rows with SWDGE dma_gather and accumulates into
its local aggregate with dma_scatter_add.  All graph structure
(indices, chunking) is baked into the NEFF at build time; the only
runtime tensors are x / W1 / b1 / W2 / b2 (+ structure-derived
constant coefficient vectors).

SWDGE descriptor generation is the bottleneck: each dma_gather /
dma_scatter_add instruction is serviced by the Q7 core pair selected
by its queue_num (cpu_id/2 == queue_num), so all chunks on one queue
serialize on 2 of the 8 GpSimd cores.  Chunks are therefore spread
over all 4 SWDGE queues.  Correctness of concurrent scatter-adds is
kept by making queues dst-disjoint: local dst d goes to queue d % 4,
so no two queues ever RMW the same aggregate row.
"""

import math
import os
import sys

import numpy as np

sys.path.insert(0, "/opt/trn_rl_repo")

NCORES = 8
NQ = 4  # SWDGE queues (ucode MAX_SWDGE_QUEUES)
BLK = 32768  # int16-addressable row window for dma_gather
G = 1024  # edges per SWDGE instruction (64 descs/lane = 1-packet max)

# full-size problem constants (hardcoded per problem spec)
N_FULL = 100_000
E_FULL = 3_200_000
F_IN = 512
HID = 64
COUT = 64
K_STEPS = 10
ALPHA = 0.1


def _plan(N, F, C, K, src, dst):
    """Host-side structural preprocessing -> per-core tensors + schedule."""
    NSH = N // NCORES
    # padded shard, multiple of 512, strictly > NSH so the last NQ rows are
    # padding rows usable as per-queue dummy-scatter targets
    PSH = ((NSH + 512) // 512) * 512
    NP = PSH * NCORES
    nblk = (NP + BLK - 1) // BLK

    deg = np.bincount(dst, minlength=N).astype(np.float64) + 1.0
    dinv = (1.0 / np.sqrt(deg)).astype(np.float32)

    core_of = dst // NSH
    # per (core, queue, block) int16 gather/scatter index lists
    gi_mqb = [[[None] * nblk for _ in range(NQ)] for _ in range(NCORES)]
    si_mqb = [[[None] * nblk for _ in range(NQ)] for _ in range(NCORES)]
    maxdeg = np.zeros((NQ, nblk), dtype=np.int64)
    maxcnt = np.zeros((NQ, nblk), dtype=np.int64)
    for m in range(NCORES):
        sel = np.nonzero(core_of == m)[0]
        s = src[sel]
        d = (dst[sel] - m * NSH).astype(np.int64)
        sp = (s // NSH) * PSH + (s % NSH)  # padded global src id
        blk = sp // BLK
        q_of = d % NQ  # dst-class -> queue (dst-disjoint across queues)
        for q in range(NQ):
            for b in range(nblk):
                bm = (blk == b) & (q_of == q)
                gi = (sp[bm] - b * BLK).astype(np.int16)
                si = d[bm].astype(np.int16)
                # sort by dst (groups same-dst edges for the chunk coloring)
                o = np.argsort(si, kind="stable")
                gi_mqb[m][q][b] = gi[o]
                si_mqb[m][q][b] = si[o]
                maxcnt[q, b] = max(maxcnt[q, b], len(si))
                if len(si):
                    maxdeg[q, b] = max(
                        maxdeg[q, b], np.bincount(si[o].astype(np.int64)).max())

    # uniform chunk schedule across cores (same NEFF on all 8).
    # dma_scatter_add loses updates when one instruction carries duplicate
    # dst indices (RMW race across SDMA engines), so every real dst must
    # appear at most once per chunk: nch >= max per-dst degree, and each
    # dst's edges are spread round-robin over chunks.
    nch = [[max(1, int(maxdeg[q][b]), int((maxcnt[q][b] + G - 1) // G))
            for b in range(nblk)] for q in range(NQ)]
    colored = [[[None] * nblk for _ in range(NQ)] for _ in range(NCORES)]
    for q in range(NQ):
        for b in range(nblk):
            while True:
                ok = True
                for m in range(NCORES):
                    gi = gi_mqb[m][q][b]
                    si = si_mqb[m][q][b]
                    nc_b = nch[q][b]
                    chunks_g = [[] for _ in range(nc_b)]
                    chunks_s = [[] for _ in range(nc_b)]
                    if len(si):
                        si64 = si.astype(np.int64)
                        grp_start = np.r_[0, np.nonzero(np.diff(si64))[0] + 1]
                        j_in_grp = np.arange(len(si64)) - np.repeat(
                            grp_start, np.diff(np.r_[grp_start, len(si64)]))
                        cid = (si64 + j_in_grp) % nc_b
                        for c in range(nc_b):
                            cm = cid == c
                            chunks_g[c] = gi[cm]
                            chunks_s[c] = si[cm]
                        if max(len(cg) for cg in chunks_g) > G:
                            ok = False
                            break
                    colored[m][q][b] = (chunks_g, chunks_s)
                if ok:
                    break
                nch[q][b] += 1
    tot_ch = [sum(nch[q]) for q in range(NQ)]

    # pack idx arrays per queue: [128, tot_ch*(G//16)] int16 -- chunk ci at
    # cols ci*(G//16)..; each chunk's [16, G//16] block (element i of chunk
    # at [i%16, i//16]) is replicated 8x down the partitions (one copy per
    # GPSIMD Q7 core, per the dma_gather contract)
    gidx_t = [np.zeros((NCORES, 128, tot_ch[q] * (G // 16)), dtype=np.int16)
              for q in range(NQ)]
    sidx_t = [np.zeros((NCORES, 128, tot_ch[q] * (G // 16)), dtype=np.int16)
              for q in range(NQ)]
    for m in range(NCORES):
        for q in range(NQ):
            ci = 0
            for b in range(nblk):
                chunks_g, chunks_s = colored[m][q][b]
                for c in range(nch[q][b]):
                    gi = np.asarray(chunks_g[c], dtype=np.int16)
                    si = np.asarray(chunks_s[c], dtype=np.int16)
                    # sort chunk by src row for HBM locality
                    o = np.argsort(gi, kind="stable")
                    gi, si = gi[o], si[o]
                    # pad with dummy pairs: gather row 0 of this block (real,
                    # finite data), scatter to this queue's pad row (never
                    # output; duplicate dummies race only within the queue)
                    pad = G - len(gi)
                    gi = np.concatenate([gi, np.zeros(pad, dtype=np.int16)])
                    si = np.concatenate(
                        [si, np.full(pad, PSH - NQ + q, dtype=np.int16)])
                    gc = gi.reshape(G // 16, 16).T  # [16, G/16]
                    sc = si.reshape(G // 16, 16).T
                    col = ci * (G // 16)
                    gidx_t[q][m, :, col:col + G // 16] = np.tile(gc, (8, 1))
                    sidx_t[q][m, :, col:col + G // 16] = np.tile(sc, (8, 1))
                    ci += 1

    # per-node coefficient vectors, tile-major [128, PSH/128]
    T = PSH // 128
    dinv_t = np.zeros((NCORES, 128, T), dtype=np.float32)
    avec_t = np.zeros((NCORES, 128, T), dtype=np.float32)
    bvec_t = np.zeros((NCORES, 128, T), dtype=np.float32)
    for m in range(NCORES):
        dl = np.zeros(PSH, dtype=np.float32)
        dl[:NSH] = dinv[m * NSH:(m + 1) * NSH]
        dinv_t[m] = dl.reshape(T, 128).T
        avec_t[m] = (0.9 * dl).reshape(T, 128).T
        bvec_t[m] = (0.9 * dl * dl).reshape(T, 128).T

    return dict(
        NSH=NSH, PSH=PSH, NP=NP, nblk=nblk, nch=nch, tot_ch=tot_ch, T=T,
        gidx=gidx_t, sidx=sidx_t, dinv=dinv_t, avec=avec_t, bvec=bvec_t,
    )


def _build(plan, F, C, K):
    """Build the SPMD Bass program (same NEFF on all 8 cores).

    Uses several sequential TileContexts: each context allocates and
    frees its own semaphores, keeping cumulative semaphore thresholds
    under the 16-bit ISA limit.  Persistent state lives in raw SBUF
    tensors / Internal DRAM tensors that outlive the contexts.
    """
    from concourse import bacc, bass, mybir, tile
    from concourse.masks import make_identity

    PSH, NP, nblk = plan["PSH"], plan["NP"], plan["nblk"]
    T = plan["T"]
    nch, tot_ch = plan["nch"], plan["tot_ch"]
    f32 = mybir.dt.float32
    i16 = mybir.dt.int16
    KT = F // 128  # contraction tiles for W1
    NB = PSH // 512  # 512-node MLP blocks

    nc = bacc.Bacc("TRN2", target_bir_lowering=False, debug=False,
                   num_devices=NCORES, num_swdge_queues=NQ)

    xT_d = nc.dram_tensor("xT", [F, PSH], f32, kind="ExternalInput").ap()
    w1t_d = nc.dram_tensor("W1T", [F, HID], f32, kind="ExternalInput").ap()
    b1_d = nc.dram_tensor("b1c", [HID, 1], f32, kind="ExternalInput").ap()
    w2t_d = nc.dram_tensor("W2T", [HID, C], f32, kind="ExternalInput").ap()
    b2_d = nc.dram_tensor("b2c", [C, 1], f32, kind="ExternalInput").ap()
    dinv_d = nc.dram_tensor("dinv", [128, T], f32, kind="ExternalInput").ap()
    avec_d = nc.dram_tensor("avec", [128, T], f32, kind="ExternalInput").ap()
    bvec_d = nc.dram_tensor("bvec", [128, T], f32, kind="ExternalInput").ap()
    gidx_d = [nc.dram_tensor(f"gidx{q}", [128, tot_ch[q] * (G // 16)], i16,
                             kind="ExternalInput").ap() for q in range(NQ)]
    sidx_d = [nc.dram_tensor(f"sidx{q}", [128, tot_ch[q] * (G // 16)], i16,
                             kind="ExternalInput").ap() for q in range(NQ)]
    out_d = nc.dram_tensor("out", [PSH, C], f32, kind="ExternalOutput").ap()

    # persistent DRAM scratch
    zs_shard = nc.dram_tensor("zs_shard", [PSH, C], f32, kind="Internal").ap()
    zs_full = nc.dram_tensor("zs_full", [nblk * BLK, C], f32,
                             kind="Internal").ap()
    agg_dr = nc.dram_tensor("agg_dr", [PSH, C], f32, kind="Internal").ap()

    # persistent SBUF state + constants (outlive the TileContexts)
    z_sb = nc.alloc_sbuf_tensor("z_sb", [128, T, C], f32).ap()
    h01_sb = nc.alloc_sbuf_tensor("h01_sb", [128, T, C], f32).ap()
    zs_sb = nc.alloc_sbuf_tensor("zs_sb", [128, T, C], f32).ap()
    agg_sb = nc.alloc_sbuf_tensor("agg_sb", [128, T, C], f32).ap()
    w1t_sb = nc.alloc_sbuf_tensor("w1t_sb", [128, KT, HID], f32).ap()
    w2t_sb = nc.alloc_sbuf_tensor("w2t_sb", [HID, C], f32).ap()
    b1_sb = nc.alloc_sbuf_tensor("b1_sb", [HID, 1], f32).ap()
    b2_sb = nc.alloc_sbuf_tensor("b2_sb", [C, 1], f32).ap()
    dinv_sb = nc.alloc_sbuf_tensor("dinv_sb", [128, T], f32).ap()
    avec_sb = nc.alloc_sbuf_tensor("avec_sb", [128, T], f32).ap()
    bvec_sb = nc.alloc_sbuf_tensor("bvec_sb", [128, T], f32).ap()
    ident = nc.alloc_sbuf_tensor("ident", [128, 128], f32).ap()
    zero_sb = nc.alloc_sbuf_tensor("zero_sb", [128, 64], f32).ap()

    dinv_b = dinv_sb.unsqueeze(2).to_broadcast([128, T, C])
    avec_b = avec_sb.unsqueeze(2).to_broadcast([128, T, C])
    bvec_b = bvec_sb.unsqueeze(2).to_broadcast([128, T, C])
    zsf_dst = zs_shard.rearrange("(t p) c -> p t c", p=128)
    agg_src = agg_dr.rearrange("(t p) c -> p t c", p=128)

    # ---- context 1: constants + MLP ----
    with tile.TileContext(nc) as tc:
        with (
            tc.tile_pool(name="xin", bufs=2) as xin,
            tc.tile_pool(name="mlps", bufs=2) as mlps,
            tc.tile_pool(name="psum", bufs=2, space="PSUM") as psum,
            tc.tile_pool(name="psumt", bufs=2, space="PSUM") as psumt,
        ):
            for t in range(KT):
                nc.sync.dma_start(w1t_sb[:, t, :], w1t_d[t * 128:(t + 1) * 128, :])
            nc.sync.dma_start(w2t_sb, w2t_d[:])
            nc.sync.dma_start(b1_sb, b1_d[:])
            nc.sync.dma_start(b2_sb, b2_d[:])
            nc.sync.dma_start(dinv_sb, dinv_d[:])
            nc.sync.dma_start(avec_sb, avec_d[:])
            nc.sync.dma_start(bvec_sb, bvec_d[:])
            make_identity(nc, ident)
            nc.vector.memset(zero_sb, 0.0)

            for nb in range(NB):
                xb = xin.tile([128, KT, 512], f32, tag="xb")
                for t in range(KT):
                    nc.sync.dma_start(
                        xb[:, t, :],
                        xT_d[t * 128:(t + 1) * 128, nb * 512:(nb + 1) * 512],
                    )
                ph = psum.tile([HID, 512], f32, tag="ph")
                for t in range(KT):
                    nc.tensor.matmul(ph[:], w1t_sb[:, t, :], xb[:, t, :],
                                     start=(t == 0), stop=(t == KT - 1))
                hT = mlps.tile([HID, 512], f32, tag="hT")
                nc.scalar.activation(hT[:], ph[:],
                                     mybir.ActivationFunctionType.Relu,
                                     bias=b1_sb[:, :1], scale=1.0)
                ph2 = psum.tile([C, 512], f32, tag="ph2")
                nc.tensor.matmul(ph2[:], w2t_sb, hT[:], start=True, stop=True)
                h2T = mlps.tile([C, 512], f32, tag="h2T")
                nc.scalar.activation(h2T[:], ph2[:],
                                     mybir.ActivationFunctionType.Copy,
                                     bias=0.0, scale=1.0)
                nc.vector.tensor_scalar_add(h2T[:], h2T[:], b2_sb[:, :1])
                for j in range(4):
                    pt = psumt.tile([128, C], f32, tag="pt")
                    nc.tensor.transpose(pt[:], h2T[:, j * 128:(j + 1) * 128],
                                        ident[:C, :C])
                    tt = nb * 4 + j
                    nc.vector.tensor_copy(z_sb[:, tt, :], pt[:])
                    nc.scalar.activation(h01_sb[:, tt, :], pt[:],
                                         mybir.ActivationFunctionType.Copy,
                                         bias=0.0, scale=ALPHA)

    # per-queue flat chunk schedule: chunk position -> block
    flat = [[b for b in range(nblk) for _ in range(nch[q][b])]
            for q in range(NQ)]
    col0 = []  # per queue: block -> starting column in idx arrays
    for q in range(NQ):
        cols, acc = [], 0
        for b in range(nblk):
            cols.append(acc)
            acc += nch[q][b] * (G // 16)
        col0.append(cols)
    max_pos = max(len(f) for f in flat)

    # ---- propagation: one context per STEPS_PER_CTX steps ----
    SPC = 2
    for s0 in range(0, K, SPC):
        with tile.TileContext(nc) as tc:
            with tc.tile_pool(name="gat", bufs=2) as gat:
                for s in range(s0, min(s0 + SPC, K)):
                    nc.vector.tensor_tensor(zs_sb, z_sb, dinv_b,
                                            op=mybir.AluOpType.mult)
                    nc.sync.dma_start(zsf_dst, zs_sb)
                    nc.gpsimd.collective_compute(
                        "AllGather", mybir.AluOpType.bypass,
                        replica_groups=[list(range(NCORES))],
                        ins=[zs_shard.opt()],
                        outs=[zs_full[:NP, :].opt()],
                    )
                    nc.sync.dma_start(
                        agg_src,
                        zero_sb.unsqueeze(1).to_broadcast([128, T, C]),
                    )
                    gi_t = [None] * NQ
                    si_t = [None] * NQ
                    blk_cur = [-1] * NQ
                    coff = [0] * NQ  # chunk offset within current block
                    for pos in range(max_pos):
                        for q in range(NQ):
                            if pos >= len(flat[q]):
                                continue
                            b = flat[q][pos]
                            if b != blk_cur[q]:
                                ncols = nch[q][b] * (G // 16)
                                gi_t[q] = gat.tile([128, ncols], i16,
                                                   name=f"gi_t{q}",
                                                   tag=f"gi{q}", bufs=1)
                                nc.sync.dma_start(
                                    gi_t[q][:],
                                    gidx_d[q][:, col0[q][b]:col0[q][b] + ncols])
                                si_t[q] = gat.tile([128, ncols], i16,
                                                   name=f"si_t{q}",
                                                   tag=f"si{q}", bufs=1)
                                nc.sync.dma_start(
                                    si_t[q][:],
                                    sidx_d[q][:, col0[q][b]:col0[q][b] + ncols])
                                blk_cur[q] = b
                                coff[q] = 0
                            cc = coff[q] * (G // 16)
                            gt = gat.tile([128, G // 128, C], f32,
                                          tag=f"gt{q}", bufs=2)
                            nc.gpsimd.dma_gather(
                                gt[:],
                                zs_full[b * BLK:(b + 1) * BLK, :],
                                gi_t[q][:, cc:cc + G // 16],
                                G, G, C, queue_num=q,
                            )
                            nc.gpsimd.dma_scatter_add(
                                agg_dr[:],
                                gt[:],
                                si_t[q][:, cc:cc + G // 16],
                                G, G, C, queue_num=q,
                            )
                            coff[q] += 1
                    nc.sync.dma_start(agg_sb, agg_src)
                    nc.vector.tensor_tensor(agg_sb, agg_sb, avec_b,
                                            op=mybir.AluOpType.mult)
                    nc.vector.tensor_tensor(z_sb, z_sb, bvec_b,
                                            op=mybir.AluOpType.mult)
                    nc.vector.tensor_tensor(z_sb, z_sb, agg_sb,
                                            op=mybir.AluOpType.add)
                    nc.vector.tensor_tensor(z_sb, z_sb, h01_sb,
                                            op=mybir.AluOpType.add)

    # ---- final context: log_softmax + output ----
    with tile.TileContext(nc) as tc:
        with tc.tile_pool(name="fin", bufs=1) as fin:
            red = fin.tile([128, T, 1], f32)
            nc.vector.tensor_reduce(red[:], z_sb,
                                    axis=mybir.AxisListType.X,
                                    op=mybir.AluOpType.max)
            nc.vector.tensor_tensor(z_sb, z_sb,
                                    red[:].to_broadcast([128, T, C]),
                                    op=mybir.AluOpType.subtract)
            nc.scalar.activation(zs_sb, z_sb,
                                 mybir.ActivationFunctionType.Exp,
                                 bias=0.0, scale=1.0)
            nc.vector.tensor_reduce(red[:], zs_sb,
                                    axis=mybir.AxisListType.X,
                                    op=mybir.AluOpType.add)
            lse = fin.tile([128, T, 1], f32)
            nc.scalar.activation(lse[:], red[:],
                                 mybir.ActivationFunctionType.Ln,
                                 bias=0.0, scale=1.0)
            nc.vector.tensor_tensor(z_sb, z_sb,
                                    lse[:].to_broadcast([128, T, C]),
                                    op=mybir.AluOpType.subtract)
            nc.sync.dma_start(out_d.rearrange("(t p) c -> p t c", p=128),
                              z_sb)

    nc.compile()
    return nc


_CACHE = {}


def _get_compiled(key, plan, F, C, K):
    if key not in _CACHE:
        _CACHE[key] = _build(plan, F, C, K)
    return _CACHE[key]


def run(x, W1, b1, W2, b2, edge_index, N, E, F, C, K, trace=False):
    from concourse import bass_utils

    src = np.asarray(edge_index[0], dtype=np.int64)
    dst = np.asarray(edge_index[1], dtype=np.int64)
    plan = _plan(N, F, C, K, src, dst)
    NSH, PSH = plan["NSH"], plan["PSH"]

    nc = _get_compiled((N, E, F, C, K, G), plan, F, C, K)

    x = np.asarray(x, dtype=np.float32)
    xT = np.ascontiguousarray(x.T)  # [F, N]
    W1T = np.ascontiguousarray(np.asarray(W1, dtype=np.float32).T)
    W2T = np.ascontiguousarray(np.asarray(W2, dtype=np.float32).T)
    b1c = np.asarray(b1, dtype=np.float32).reshape(HID, 1)
    b2c = np.asarray(b2, dtype=np.float32).reshape(COUT, 1)

    in_maps = []
    for m in range(NCORES):
        xTs = np.zeros((F, PSH), dtype=np.float32)
        xTs[:, :NSH] = xT[:, m * NSH:(m + 1) * NSH]
        im = {
            "xT": xTs, "W1T": W1T, "b1c": b1c, "W2T": W2T, "b2c": b2c,
            "dinv": plan["dinv"][m], "avec": plan["avec"][m],
            "bvec": plan["bvec"][m],
        }
        for q in range(NQ):
            im[f"gidx{q}"] = plan["gidx"][q][m]
            im[f"sidx{q}"] = plan["sidx"][q][m]
        in_maps.append(im)

    try:
        res = bass_utils.run_bass_kernel_spmd(
            nc, in_maps, core_ids=list(range(NCORES)), trace=trace,
        )
    except ModuleNotFoundError:
        res = bass_utils.run_bass_kernel_spmd(
            nc, in_maps, core_ids=list(range(NCORES)), trace=False,
        )
    outs = res.results
    full = np.empty((N, C), dtype=np.float32)
    for m in range(NCORES):
        full[m * NSH:(m + 1) * NSH] = outs[m]["out"][:NSH]
    return full, res


def kernel(x, W1, b1, W2, b2, edge_index):
    out, _ = run(x, W1, b1, W2, b2, edge_index,
                 N=N_FULL, E=E_FULL, F=F_IN, C=COUT, K=K_STEPS)
    return out
